# revision 36
# baseline (speedup 1.0000x reference)
"""GPT-2 attention block (B=4, S=1024, D=1024, H=16) on 8 TRN2 NeuronCores.

Tensor-parallel over heads: core i holds heads 2i, 2i+1. qkv is computed
with per-core weight columns in transposed layout [cols, tokens]; v is
PE-transposed into [tokens, cols] stationary tiles. Attention scores are
built directly in transposed layout P^T[k, q] so they feed the AV matmul
as the moving operand; the softmax denominator rides along the AV matmul
as an appended ones-column block of the stationary operand (v_aug =
[v_h | 1]). Each span runs in two phases: phase 1 streams all score
matmuls, exps the two head segments (laid out adjacently around the psum
bank boundary so one ACT op covers both) into SBUF, and zeroes the
causal upper triangle of diagonal blocks via gpsimd affine_select
(exp(-1e4)==0, so post-exp zeroing matches the reference mask); phase 2
streams all AV matmuls from the SBUF tiles, so the PE never stalls on
the exp latency. qkv-chunk and c_proj work is emitted as generators and
pumped between phase-1 steps to fill the PE while ACT streams exps.
Softmax division stages the denominator to SBUF (the custom-DVE
reciprocal mis-reads PSUM on HW) and uses the single-instruction DVE
reciprocal_approx_fast (~51 ULP). The ACT engine stays exp-only: one
act-table load for the whole kernel. c_proj is fully local: each core
computes a bf16 partial over its own 128 w_proj rows for ALL tokens and
the host sums the 8 partials - no collectives, so cores are fully
decoupled. A burst of dummy matmuls on a zeroed tile warms the PE HAM
clock gate during the startup DMAs, which are issued in criticality
order on the FIFO sync HWDGE ring so the first matmul's inputs transfer
at full HBM bandwidth.
"""

from collections import deque

import numpy as np
import ml_dtypes

import concourse.bass as bass
import concourse.mybir as mybir
import concourse.tile as tile
from concourse import bacc
from concourse.bass_utils import run_bass_kernel_spmd

B, S, D, H = 4, 1024, 1024, 16
HD = D // H  # 64
NT = B * S  # 4096 tokens
N_CORES = 8
CORE_IDS = list(range(N_CORES))
BF16 = mybir.dt.bfloat16
F32 = mybir.dt.float32
AF = mybir.ActivationFunctionType

_CACHE = {}


def build_nc():
    nc = bacc.Bacc("TRN2", target_bir_lowering=False, debug=False, num_devices=N_CORES)

    xt_d = nc.dram_tensor("xt", [D, NT], BF16, kind="ExternalInput")
    wqkv_d = nc.dram_tensor("wqkv", [D, 384], BF16, kind="ExternalInput")
    bqkv_d = nc.dram_tensor("bqkv", [3, 128, 1], F32, kind="ExternalInput")
    eye_d = nc.dram_tensor("eye", [128, 128], BF16, kind="ExternalInput")
    wpown_d = nc.dram_tensor("wpown", [128, D], BF16, kind="ExternalInput")
    out_d = nc.dram_tensor("out", [D, NT], BF16, kind="ExternalOutput")

    with tile.TileContext(nc) as tc:
        with (
            tc.tile_pool(name="persist", bufs=1) as pp,
            tc.tile_pool(name="xin", bufs=4) as xp,
            tc.tile_pool(name="ptp", bufs=14) as ptp,
            tc.tile_pool(name="work", bufs=2) as wk,
            tc.tile_pool(name="outs", bufs=2) as op,
            tc.tile_pool(name="ps", bufs=2, space="PSUM") as psp,
            tc.tile_pool(name="ps_pt", bufs=2, space="PSUM") as ps_pt,
            tc.tile_pool(name="ps_at", bufs=1, space="PSUM") as ps_at,
            tc.tile_pool(name="dram", bufs=1, space="DRAM") as dp,
        ):
            # ---- HAM warmup: dummy matmuls on a zeroed tile, issued before
            # any DMA-dependent work so the PE clock gate opens to 8/8 while
            # the startup DMAs are still in flight ----
            warm = pp.tile([128, 128], BF16, tag="warm")
            nc.gpsimd.memset(warm[:], 0.0)
            warm_ps = psp.tile([128, 512], F32, tag="ps", name="warmps")
            for i in range(30):
                nc.tensor.matmul(
                    warm_ps[:, 0:128], warm[:], warm[:], start=True, stop=True
                )

            # ---- persistent weights / constants ----
            # the sync HWDGE ring transfers FIFO per issue order, so the
            # startup DMAs are issued in criticality order: wq/x0 gate the
            # first matmuls, then wkv, bias/eye, x1, and wpown (first needed
            # by c_proj tens of us in) at the back of the ring
            wqkv = pp.tile([128, 8, 384], BF16, tag="wqkv")
            wqsrc = wqkv_d.rearrange("(a p) c -> p a c", p=128)
            biast = pp.tile([128, 3], F32, tag="biast")
            bias = [biast[:, m : m + 1] for m in range(3)]
            eye = pp.tile([128, 128], BF16, tag="eye")
            wpown = pp.tile([128, D], BF16, tag="wpown")

            xbs = {}

            def alloc_xb(t):
                xb = xp.tile([128, 8, 512], BF16, tag="x", name=f"x_{t}")
                xbs[t] = (
                    xb,
                    xt_d[:, 512 * t : 512 * (t + 1)].rearrange(
                        "(a p) c -> p a c", p=128
                    ),
                )
                return xbs[t]

            def x_dma(t, g):
                xb, xsrc = xbs[t]
                nc.sync.dma_start(
                    xb[:, 4 * g : 4 * (g + 1), :], xsrc[:, 4 * g : 4 * (g + 1), :]
                )

            # startup transfers are sliced so the first matmuls are gated by
            # as few bytes as possible: wq k0-3 (128KB) + x0 k0-1 (256KB)
            # release the first two matmuls; later slices land while the PE
            # consumes earlier ones
            alloc_xb(0)
            xb0, xsrc0 = xbs[0]
            nc.sync.dma_start(wqkv[:, 0:4, 0:128], wqsrc[:, 0:4, 0:128])
            nc.sync.dma_start(xb0[:, 0:2, :], xsrc0[:, 0:2, :])
            nc.sync.dma_start(xb0[:, 2:4, :], xsrc0[:, 2:4, :])
            nc.sync.dma_start(wqkv[:, 4:8, 0:128], wqsrc[:, 4:8, 0:128])
            nc.sync.dma_start(xb0[:, 4:8, :], xsrc0[:, 4:8, :])
            nc.sync.dma_start(wqkv[:, :, 128:384], wqsrc[:, :, 128:384])
            nc.sync.dma_start(biast[:], bqkv_d.rearrange("m p c -> p (m c)"))
            nc.sync.dma_start(eye[:], eye_d[:])
            alloc_xb(1)
            x_dma(1, 0)
            x_dma(1, 1)
            # wpown is first needed by c_proj tens of us in; putting it at the
            # back of the same FIFO ring keeps the x0/wqkv transfers at full
            # HBM bandwidth instead of round-robining with a second ring
            nc.sync.dma_start(wpown[:], wpown_d[:])

            qt, kt, vt = [], [], []
            vaug = {}
            at_sb = []
            chunk_done = [False] * 8
            fill = deque()

            def pump(n=1):
                done = 0
                while fill and done < n:
                    try:
                        next(fill[0])
                        done += 1
                    except StopIteration:
                        fill.popleft()

            def need_chunk(t):
                while not chunk_done[t]:
                    assert fill, f"chunk {t} required but fill queue empty"
                    pump(1)

            def qkv_chunk_gen(t):
                if t not in xbs:
                    alloc_xb(t)
                    x_dma(t, 0)
                    x_dma(t, 1)
                xb, _ = xbs[t]
                yield
                for m, store in enumerate((qt, kt, vt)):
                    ps = psp.tile([128, 512], F32, tag="ps", name=f"qkv{m}_{t}")
                    for k in range(8):
                        nc.tensor.matmul(
                            ps[:],
                            wqkv[:, k, 128 * m : 128 * (m + 1)],
                            xb[:, k, :],
                            start=(k == 0),
                            stop=(k == 7),
                        )
                        if k % 2 == 1:
                            yield
                    sb = pp.tile(
                        [128, 512], BF16, tag=f"qkv{m}_{t}", name=f"qkv{m}_{t}"
                    )
                    if m < 2:
                        nc.vector.tensor_scalar_add(sb[:], ps[:], bias[m])
                    else:
                        # split the v copy so each PE transpose waits on 1/4
                        for i in range(4):
                            nc.vector.tensor_scalar_add(
                                sb[:, 128 * i : 128 * (i + 1)],
                                ps[:, 128 * i : 128 * (i + 1)],
                                bias[m],
                            )
                    store.append(sb)
                    yield
                # v_aug: [tokens, (v_h0 | ones | v_h1 | ones)] via PE transpose
                tp = psp.tile([128, 512], BF16, tag="ps", name=f"vt{t}")
                for i in range(4):
                    nc.tensor.transpose(
                        tp[:, 128 * i : 128 * (i + 1)],
                        vt[t][:, 128 * i : 128 * (i + 1)],
                        eye[:],
                    )
                    if i % 2 == 1:
                        yield
                for i in range(4):
                    va = pp.tile([128, 256], BF16, tag=f"va{t}_{i}", name=f"va{t}_{i}")
                    va4 = va.rearrange("p (a b) -> p a b", b=64)
                    nc.vector.tensor_copy(
                        va4[:, 0:3:2, :],
                        tp[:, 128 * i : 128 * (i + 1)].rearrange(
                            "p (a b) -> p a b", b=64
                        ),
                    )
                    nc.gpsimd.memset(va4[:, 1:4:2, :], 1.0)
                    vaug[(t, i)] = va
                    if i % 2 == 1:
                        yield
                chunk_done[t] = True

            def attention_span(b, s):
                need_chunk(2 * b + s)
                aT = at_sb[b]
                tcq = 2 * b + s
                last = 4 * s + 3
                segs = []
                # phase 1: stream all score matmuls; exp each kc's two head
                # segments (adjacent around the bank boundary) into SBUF;
                # zero the causal triangle of diagonal blocks on gpsimd
                for kc in range(last + 1):
                    off = max(0, kc * 128 - s * 512)
                    width = 512 - off
                    tck = 2 * b + kc // 4
                    kcol = (kc % 4) * 128
                    dq = kc * 128 - s * 512  # diag col in span coords
                    pt_ps = ps_pt.tile(
                        [128, 1024], F32, tag="pt", name=f"pt{b}_{s}_{kc}"
                    )
                    pt_sb = ptp.tile(
                        [128, 1024], BF16, tag="pt", name=f"ptsb{b}_{s}_{kc}"
                    )
                    c0 = (512 - width, 512)
                    for h in range(2):
                        nc.tensor.matmul(
                            pt_ps[:, c0[h] : c0[h] + width],
                            kt[tck][64 * h : 64 * h + 64, kcol : kcol + 128],
                            qt[tcq][64 * h : 64 * h + 64, off:512],
                            start=True,
                            stop=True,
                        )
                    nc.scalar.activation(
                        pt_sb[:, 512 - width : 512 + width],
                        pt_ps[:, 512 - width : 512 + width],
                        AF.Exp,
                    )
                    if dq >= 0:
                        # the diag block is always the first 128 cols of each
                        # seg; keep k_local <= q_local, zero the rest
                        for h in range(2):
                            nc.gpsimd.affine_select(
                                pt_sb[:, c0[h] : c0[h] + 128],
                                pt_sb[:, c0[h] : c0[h] + 128],
                                pattern=[[1, 128]],
                                compare_op=mybir.AluOpType.is_ge,
                                fill=0.0,
                                base=0,
                                channel_multiplier=-1,
                            )
                    segs.append((kc, off, width, vaug[(tck, kc % 4)], pt_sb, c0))
                    pump(2)
                pump(6)
                # phase 2: stream all AV matmuls back-to-back; both heads
                # accumulate into one 2-bank tile (h -> bank h)
                at2 = ps_at.tile([128, 1024], F32, tag="at", name=f"at{b}_{s}")
                for kc, off, width, va, pt_sb, c0 in segs:
                    for h in range(2):
                        nc.tensor.matmul(
                            at2[:, 512 * h + off : 512 * h + 512],
                            va[:, 128 * h : 128 * (h + 1)],
                            pt_sb[:, c0[h] : c0[h] + width],
                            start=(kc == 0),
                            stop=(kc == last),
                        )
                # softmax denominators: psum -> sbuf (the custom-DVE recip
                # mis-reads PSUM on HW), one reciprocal + per-head normalize
                den = wk.tile([64, 1024], F32, tag="den", name=f"den{b}_{s}")
                nc.vector.tensor_copy(den[:], at2[64:128, :])
                rec = wk.tile([64, 1024], F32, tag="rec", name=f"rec{b}_{s}")
                nc.vector.reciprocal_approx_fast(rec[:], den[:])
                for h in range(2):
                    nc.vector.tensor_mul(
                        aT[64 * h : 64 * h + 64, 512 * s : 512 * (s + 1)],
                        at2[0:64, 512 * h : 512 * h + 512],
                        rec[:, 512 * h : 512 * h + 512],
                    )

            def tail_psum(m, name, no_at=False):
                # round-robin psum across pools for independent matmuls
                kind = m % 4
                if kind == 1:
                    return ps_pt.tile([128, 512], F32, tag="pt", name=name)
                if kind == 2 and not no_at:
                    return ps_at.tile([128, 512], F32, tag="at", name=name)
                return psp.tile([128, 512], F32, tag="ps", name=name)

            def cproj_gen(b, h2, evac="mix", no_at=False, dma_per_m=False):
                # local partial c_proj for span h2 of batch b over this
                # core's 128 w_proj rows; host sums the 8 per-core partials.
                # evacuations alternate DVE/ACT; the 8 column blocks land in
                # one [128, 8, 512] staging tile so the span ships as 2 DMAs.
                osb = op.tile([128, 8, 512], BF16, tag=f"osb{h2}", name=f"osb{b}_{h2}")
                dst = out_d[
                    :, 1024 * b + 512 * h2 : 1024 * b + 512 * (h2 + 1)
                ].rearrange("(a p) c -> p a c", p=128)
                for m in range(8):
                    ps = tail_psum(m, f"cp{b}_{m}_{h2}", no_at=no_at)
                    nc.tensor.matmul(
                        ps[:],
                        wpown[:, 128 * m : 128 * (m + 1)],
                        at_sb[b][:, 512 * h2 : 512 * (h2 + 1)],
                        start=True,
                        stop=True,
                    )
                    if evac == "mix":
                        on_act = m % 2 == 1
                    elif evac == "act_major":
                        on_act = m < 6
                    else:  # dve_major
                        on_act = m >= 6
                    if on_act:
                        nc.scalar.activation(osb[:, m, :], ps[:], AF.Copy)
                    else:
                        nc.vector.tensor_copy(osb[:, m, :], ps[:])
                    if dma_per_m:
                        # tail call: ship every evac individually so the
                        # final transfer is as small and early as possible
                        nc.sync.dma_start(dst[:, m : m + 1, :], osb[:, m : m + 1, :])
                    elif m % 2 == 1:
                        # ship each evac pair as soon as it lands so the
                        # final transfer starts early (shorter kernel tail)
                        nc.sync.dma_start(
                            dst[:, m - 1 : m + 1, :], osb[:, m - 1 : m + 1, :]
                        )
                    yield

            # ---- program ----
            for b in range(B):
                aT = pp.tile([128, 1024], BF16, tag=f"aT{b}", name=f"aT{b}")
                at_sb.append(aT)
            for _ in qkv_chunk_gen(0):
                pass
            for _ in qkv_chunk_gen(1):
                pass
            fill.append(qkv_chunk_gen(2))
            attention_span(0, 0)
            fill.append(qkv_chunk_gen(3))
            attention_span(0, 1)
            fill.append(cproj_gen(0, 0))
            fill.append(qkv_chunk_gen(4))
            attention_span(1, 0)
            fill.append(cproj_gen(0, 1))
            fill.append(qkv_chunk_gen(5))
            attention_span(1, 1)
            fill.append(cproj_gen(1, 0))
            fill.append(qkv_chunk_gen(6))
            attention_span(2, 0)
            fill.append(cproj_gen(1, 1))
            fill.append(qkv_chunk_gen(7))
            attention_span(2, 1)
            fill.append(cproj_gen(2, 0))
            attention_span(3, 0)
            fill.append(cproj_gen(2, 1))
            attention_span(3, 1)
            while fill:
                pump(100)
            # cproj(3,0) is held back so its matmuls cover the PE gap while
            # span(3,1)'s reciprocal chain runs on the DVE: its evacs go to
            # the (idle) ACT engine and its psum avoids the `at` tag so
            # nothing in it waits on that chain; cproj(3,1) then evacuates on
            # the DVE, which is free once the reciprocal chain completes
            for _ in cproj_gen(3, 0, evac="act_major", no_at=True):
                pass
            for _ in cproj_gen(3, 1, evac="dve_major", no_at=True, dma_per_m=True):
                pass

    nc.compile()
    return nc


def _prep_inputs(x, w_attn, b_attn, w_proj):
    bf = ml_dtypes.bfloat16
    xt = np.ascontiguousarray(x.reshape(NT, D).T).astype(bf)
    scale = 1.0 / np.sqrt(np.float32(HD))
    wp = w_proj.astype(bf)
    eye = np.eye(128, dtype=np.float32).astype(bf)
    in_maps = []
    for i in range(N_CORES):
        cc = 128 * i
        wq = (w_attn[:, cc : cc + 128] * scale).astype(bf)
        wkk = w_attn[:, D + cc : D + cc + 128].astype(bf)
        wv = w_attn[:, 2 * D + cc : 2 * D + cc + 128].astype(bf)
        wqkv = np.concatenate([wq, wkk, wv], axis=1)
        bqkv = np.stack(
            [
                (b_attn[cc : cc + 128] * scale).astype(np.float32),
                b_attn[D + cc : D + cc + 128].astype(np.float32),
                b_attn[2 * D + cc : 2 * D + cc + 128].astype(np.float32),
            ]
        ).reshape(3, 128, 1)
        in_maps.append(
            {
                "xt": xt,
                "wqkv": wqkv,
                "bqkv": bqkv,
                "wpown": np.ascontiguousarray(wp[cc : cc + 128, :]),
                "eye": eye,
            }
        )
    return in_maps


def _bf16_to_f32(a):
    # fast vectorized upcast: bf16 is the top 16 bits of f32
    return (a.view(np.uint16).astype(np.uint32) << 16).view(np.float32)


def run_on_hw(in_maps, trace=False, **kw):
    if "nc" not in _CACHE:
        _CACHE["nc"] = build_nc()
    return run_bass_kernel_spmd(_CACHE["nc"], in_maps, CORE_IDS, trace=trace, **kw)


def assemble_output(results, b_proj):
    # every core returns a bf16 partial [D, NT] over its 128 w_proj rows;
    # the sum over cores is the c_proj contraction
    outT = _bf16_to_f32(results[0]["out"])
    for j in range(1, N_CORES):
        outT += _bf16_to_f32(results[j]["out"])
    return (outT.T + b_proj[None, :].astype(np.float32)).reshape(B, S, D)


def kernel(x, w_attn, b_attn, w_proj, b_proj):
    in_maps = _prep_inputs(
        np.asarray(x, dtype=np.float32),
        np.asarray(w_attn, dtype=np.float32),
        np.asarray(b_attn, dtype=np.float32),
        np.asarray(w_proj, dtype=np.float32),
    )
    res = run_on_hw(in_maps)
    return assemble_output(res.results, np.asarray(b_proj, dtype=np.float32))
